# revision 25
# baseline (speedup 1.0000x reference)
"""Trainium2 Bass kernel for CustomEmbedding lookup.

Reference semantics:
    table = where(is_num[:, None], sin(num_value/1000 * (arange(D)+1)), weight)
    out   = table[x]                    # x: (8, 4096) int32, table: (50000, 512) f32

Strategy (8 NeuronCores, SPMD, memory-bound):
  - Host: materialize the merged static table (constant at init) and keep a
    bf16 copy (tolerance is 2e-2; bf16 rounding is ~2e-3 of max).
  - Shard x across the 8 cores by batch row (4096 tokens/core); replicate
    the 50 MB bf16 table into each core's HBM.
  - Device (per core), default arch "gb16": host-compacted int16 index
    streams (lo/hi halves of the vocab, since dma_gather indices are int16),
    chunked dma_gather of bf16 rows (1 KB each) into SBUF, ACT-engine
    copy-cast to f32, HWDGE stores of the compacted f32 stream. Host
    unpermutes rows back to token positions (established baseline pattern).
    Measured: ~63.8 us (vs 66.4 us for the f32 indirect baseline).
  - Measured HW facts that shape this design (from perfetto traces here):
    SDMA engines move random 2 KB gather descriptors at ~82 ns (line rate,
    not latency-bound); dma_gather desc-gen costs ~4-5 ns/row on Pool but
    pays a ~9 us mlp-ucode library reload before the first gather;
    indirect_dma_start needs no reload but costs ~11 ns/row of Pool time
    (1.4 us per 128-row instruction incl. dispatch gaps) - it is
    desc-gen-bound end to end. Stores run at line rate (~26 GB/s/engine,
    8 KB descriptors). bf16 halves the random-read bytes.
  - Fragile configs found by experiment (each hangs the device DMA path;
    keep to the proven config): dma_gather chunks with c=1 (128 rows),
    all chunks on one SWDGE queue / num_swdge_queues=1, tile-pool bufs
    above 4(gather)+3(store) (exceeds the 8 DMA completion sem lanes),
    DVE tensor_copy concurrent with gather desc-gen.
  - EMB_KERNEL_ARCH=indirect keeps the previous f32 indirect-DMA
    implementation (~66 us); kernel() also auto-falls-back to it if the
    gather path raises.
  - EMB_TABLE_DTYPE=fp8 gathers a float8_e4m3 table (512 B rows) and
    patches the ~1k worst-quantized rows' tokens exactly on the host.
"""

import os

import numpy as np

# Problem shape (hardcoded per harness contract).
N_CORES = 8
B, S = 8, 4096          # x shape
V, D = 50000, 512       # table shape
P = 128                 # SBUF partitions
S_CORE = (B * S) // N_CORES   # tokens per core = 4096
T = S_CORE // P         # tokens per partition = 32
HALF = 32768            # int16-addressable row limit

# Static capacities for the two compacted streams (multiples of 128).
# Uniform x: nLo ~ B(4096, .655) => mean 2685, sigma ~30. Caps clear the
# mean by >4 sigma; a host-side fallback handles any overflow exactly.
LO_CAP = 2816
HI_CAP = 1536
CHUNK = 512             # rows per dma_gather

_PROGS = {}
LAST_RESULTS = None  # BassKernelResults of the last run (for test harness)
TRACE = False


def _install_ntff_hook():
    """Provide antenv.axon_hooks (absent on this image) so
    run_bass_kernel_spmd(trace=True) can capture NTFF profiles."""
    import sys
    import types

    if "antenv.axon_hooks" in sys.modules:
        return
    mod = types.ModuleType("antenv.axon_hooks")
    state = {"hook": None}
    mod.set_axon_ntff_profile_hook = lambda h: state.update(hook=h)
    mod.get_axon_ntff_profile_hook = lambda: state["hook"]
    sys.modules["antenv.axon_hooks"] = mod
    import antenv

    antenv.axon_hooks = mod
    from trn_agent_boot.trn_boot import _ntff_profile_via_ctypes

    mod.set_axon_ntff_profile_hook(
        _ntff_profile_via_ctypes("/opt/axon/libaxon_pjrt.so"))


def _chunks_sized(sizes):
    out = []
    base = 0
    for n in sizes:
        out.append((base, n))
        base += n
    return out


# Ramped chunk sizes: small first chunks complete quickly (prime the
# cast/store pipeline through the 4-queue packet round-robin), big
# steady-state chunks amortize desc-gen, small tail drains fast.
LO_SIZES = [512, 512, 512, 512, 512, 256]   # = LO_CAP
HI_SIZES = [512, 512, 512]                  # = HI_CAP


def _build_nc_gb16(tdt_name="bf16"):
    """Quantized table, chunked dma_gather -> ACT/DVE cast -> HWDGE store."""
    import concourse.bacc as bacc
    import concourse.mybir as mybir
    import concourse.tile as tile

    tdt = mybir.dt.bfloat16 if tdt_name == "bf16" else mybir.dt.float8e4
    nc = bacc.Bacc("TRN2", target_bir_lowering=False, debug=False,
                   num_devices=N_CORES, num_swdge_queues=4)
    table = nc.dram_tensor("table", [V, D], tdt,
                           kind="ExternalInput").ap()
    idx_lo = nc.dram_tensor("idxLo", [P, LO_CAP // 16], mybir.dt.int16,
                            kind="ExternalInput").ap()
    idx_hi = nc.dram_tensor("idxHi", [P, HI_CAP // 16], mybir.dt.int16,
                            kind="ExternalInput").ap()
    out_lo = nc.dram_tensor("outLo", [LO_CAP, D], mybir.dt.float32,
                            kind="ExternalOutput").ap()
    out_hi = nc.dram_tensor("outHi", [HI_CAP, D], mybir.dt.float32,
                            kind="ExternalOutput").ap()
    # DRAM row p*C + c holds stream element c*128 + p (dma_gather layout);
    # per-partition rows are contiguous so store descriptors are big.
    lo_v = out_lo.rearrange("(p c) d -> p c d", p=P)
    hi_v = out_hi.rearrange("(p c) d -> p c d", p=P)

    assert sum(LO_SIZES) == LO_CAP and sum(HI_SIZES) == HI_CAP
    chunks = ([("lo", b, n) for b, n in _chunks_sized(LO_SIZES)]
              + [("hi", b, n) for b, n in _chunks_sized(HI_SIZES)])

    with tile.TileContext(nc) as tc:
        with tc.tile_pool(name="idx", bufs=1) as idxp, \
             tc.tile_pool(name="bfrows", bufs=4) as bfp, \
             tc.tile_pool(name="f32rows", bufs=3) as f32p:
            lo_sb = idxp.tile([P, LO_CAP // 16], mybir.dt.int16, tag="ilo")
            hi_sb = idxp.tile([P, HI_CAP // 16], mybir.dt.int16, tag="ihi")
            nc.sync.dma_start(out=lo_sb[:], in_=idx_lo[:, :])
            nc.scalar.dma_start(out=hi_sb[:], in_=idx_hi[:, :])
            for k, (kind, cbase, n) in enumerate(chunks):
                src = table[:HALF, :] if kind == "lo" else table[HALF:, :]
                isb = lo_sb if kind == "lo" else hi_sb
                odr = lo_v if kind == "lo" else hi_v
                c = n // P
                rows_bf = bfp.tile([P, c * D], mybir.dt.bfloat16, tag="bf")
                nc.gpsimd.dma_gather(
                    out_ap=rows_bf[:].rearrange("p (c d) -> p c d", d=D),
                    in_ap=src,
                    idxs_ap=isb[:, cbase // 16:(cbase + n) // 16],
                    num_idxs=n,
                    num_idxs_reg=n,
                    elem_size=D,
                    single_packet=True,
                    queue_num=k % 4,
                )
                rows_f = f32p.tile([P, c * D], mybir.dt.float32, tag="f32")
                # Cast in halves so downstream stores start sooner.
                hw = max(1, c // 2) * D
                for h in range(0, c * D, hw):
                    nc.scalar.copy(out=rows_f[:, h:h + hw],
                                   in_=rows_bf[:, h:h + hw])
                nc.sync.dma_start(
                    out=odr[:, cbase // P:(cbase + n) // P, :],
                    in_=rows_f[:].rearrange("p (c d) -> p c d", d=D),
                )
    nc.compile()
    return nc


def _build_nc_indirect():
    """Fallback: 32x int32 indirect DMAs (one index per partition each)."""
    import concourse.bacc as bacc
    import concourse.bass as bass
    import concourse.mybir as mybir
    import concourse.tile as tile

    nc = bacc.Bacc("TRN2", target_bir_lowering=False, debug=False,
                   num_devices=N_CORES)
    xs = nc.dram_tensor("xs", [S_CORE], mybir.dt.int32,
                        kind="ExternalInput").ap()
    table = nc.dram_tensor("table", [V, D], mybir.dt.float32,
                           kind="ExternalInput").ap()
    out = nc.dram_tensor("out", [S_CORE, D], mybir.dt.float32,
                         kind="ExternalOutput").ap()

    GW = 4
    NW = T // GW
    with tile.TileContext(nc) as tc:
        with tc.tile_pool(name="idx", bufs=1) as idxp, \
             tc.tile_pool(name="rows", bufs=4) as rowp:
            xv = xs.rearrange("(p t) -> p t", p=P)
            idx_sb = idxp.tile([P, T], mybir.dt.int32)
            nc.sync.dma_start(out=idx_sb[:, :GW], in_=xv[:, :GW])
            nc.scalar.dma_start(out=idx_sb[:, GW:], in_=xv[:, GW:])
            outv = out.rearrange("(p t) d -> p t d", p=P)
            for w in range(NW):
                rows = rowp.tile([P, GW * D], mybir.dt.float32)
                for j in range(GW):
                    t = w * GW + j
                    nc.gpsimd.indirect_dma_start(
                        out=rows[:, j * D:(j + 1) * D],
                        out_offset=None,
                        in_=table[:],
                        in_offset=bass.IndirectOffsetOnAxis(
                            ap=idx_sb[:, t:t + 1], axis=0),
                    )
                if w < NW - 1:
                    eng = nc.sync if w % 2 == 0 else nc.scalar
                    eng.dma_start(
                        out=outv[:, w * GW:(w + 1) * GW, :],
                        in_=rows[:].rearrange("p (t d) -> p t d", d=D),
                    )
                else:
                    for j in range(GW):
                        t = w * GW + j
                        eng = nc.sync if j % 2 == 0 else nc.scalar
                        eng.dma_start(
                            out=outv[:, t, :],
                            in_=rows[:, j * D:(j + 1) * D],
                        )
    nc.compile()
    return nc


def _get_prog(arch):
    if arch not in _PROGS:
        if arch.startswith("gb16"):
            _PROGS[arch] = _build_nc_gb16(arch.split("-")[1])
        elif arch == "ib16":
            _PROGS[arch] = _build_nc_ib16()
        else:
            _PROGS[arch] = _build_nc_indirect()
    return _PROGS[arch]


def _merged_table(weight, num_value, is_num):
    """Merged static table: sinusoid rows where is_num, else weight."""
    table = np.array(weight, dtype=np.float32, copy=True)
    rows = np.nonzero(np.asarray(is_num))[0]
    if rows.size:
        freqs = np.arange(1, D + 1, dtype=np.float32)
        scaled = np.asarray(num_value)[rows].astype(np.float32) / np.float32(1000.0)
        table[rows] = np.sin(scaled[:, None] * freqs[None, :]).astype(np.float32)
    return table


def _wrap16(stream, cap):
    """stream (cap,) int16 -> [128, cap/16]: index i at [i%16, i//16],
    replicated across the 8 GpSimd core partition groups."""
    t = np.ascontiguousarray(stream.reshape(cap // 16, 16).T)
    return np.tile(t, (8, 1))


def _kernel_gb16(x, table, tdt_name="bf16"):
    import ml_dtypes
    from concourse.bass_utils import run_bass_kernel_spmd

    nc = _get_prog("gb16-" + tdt_name)
    np_tdt = (ml_dtypes.bfloat16 if tdt_name == "bf16"
              else ml_dtypes.float8_e4m3)
    table_bf = table.astype(np_tdt)
    xs = np.asarray(x, dtype=np.int32).reshape(N_CORES, S_CORE)
    # Rows whose quantization error is too large get patched exactly on
    # the host after the device pass (rare-row exception path).
    bad_rows = None
    if tdt_name == "fp8":
        err_row = np.abs(table - table_bf.astype(np.float32)).max(axis=1)
        denom = np.abs(table[np.unique(xs)]).max()
        bad = err_row > 0.8 * 0.02 * denom
        bad_rows = np.zeros(V, dtype=bool)
        bad_rows[np.nonzero(bad)[0]] = True
    in_maps = []
    pos = []
    for c in range(N_CORES):
        xc = xs[c]
        lo_pos = np.nonzero(xc < HALF)[0]
        hi_pos = np.nonzero(xc >= HALF)[0]
        pos.append((lo_pos, hi_pos))
        s_lo = np.full(LO_CAP, -1, dtype=np.int16)
        s_hi = np.full(HI_CAP, -1, dtype=np.int16)
        n_lo = min(lo_pos.size, LO_CAP)
        n_hi = min(hi_pos.size, HI_CAP)
        s_lo[:n_lo] = xc[lo_pos[:n_lo]].astype(np.int16)
        s_hi[:n_hi] = (xc[hi_pos[:n_hi]] - HALF).astype(np.int16)
        in_maps.append({"table": table_bf,
                        "idxLo": _wrap16(s_lo, LO_CAP),
                        "idxHi": _wrap16(s_hi, HI_CAP)})

    res = run_bass_kernel_spmd(nc, in_maps, core_ids=list(range(N_CORES)),
                               trace=TRACE)
    out = np.empty((N_CORES, S_CORE, D), dtype=np.float32)
    for c in range(N_CORES):
        lo_pos, hi_pos = pos[c]
        r = res.results[c]
        n_lo = min(lo_pos.size, LO_CAP)
        n_hi = min(hi_pos.size, HI_CAP)
        # DRAM row p*C + c holds stream element c*128 + p: transpose back.
        lo_rows = r["outLo"].reshape(P, LO_CAP // P, D).transpose(1, 0, 2)
        hi_rows = r["outHi"].reshape(P, HI_CAP // P, D).transpose(1, 0, 2)
        out[c][lo_pos[:n_lo]] = lo_rows.reshape(LO_CAP, D)[:n_lo]
        out[c][hi_pos[:n_hi]] = hi_rows.reshape(HI_CAP, D)[:n_hi]
        # Exact host fallback for (statistically impossible) cap overflow.
        for ps, n_cap in ((lo_pos, n_lo), (hi_pos, n_hi)):
            if ps.size > n_cap:
                ovf = ps[n_cap:]
                out[c][ovf] = table[xs[c][ovf]]
        if bad_rows is not None:
            pos_bad = np.nonzero(bad_rows[xs[c]])[0]
            if pos_bad.size:
                out[c][pos_bad] = table[xs[c][pos_bad]]
    return res, out


def _kernel_indirect(x, table, arch="indirect"):
    from concourse.bass_utils import run_bass_kernel_spmd

    if arch == "ib16":
        import ml_dtypes
        table = table.astype(ml_dtypes.bfloat16)
    nc = _get_prog(arch)
    xflat = np.ascontiguousarray(np.asarray(x, dtype=np.int32).reshape(-1))
    in_maps = [
        {"xs": xflat[c * S_CORE:(c + 1) * S_CORE], "table": table}
        for c in range(N_CORES)
    ]
    res = run_bass_kernel_spmd(nc, in_maps, core_ids=list(range(N_CORES)),
                               trace=TRACE)
    out = np.stack([r["out"] for r in res.results])
    return res, out


def kernel(x, weight, num_value, is_num):
    global LAST_RESULTS
    if TRACE:
        _install_ntff_hook()

    table = _merged_table(weight, num_value, is_num)
    arch = os.environ.get("EMB_KERNEL_ARCH", "gb16")
    try:
        if arch in ("indirect", "ib16"):
            res, out = _kernel_indirect(x, table, arch)
        else:
            res, out = _kernel_gb16(x, table,
                                    os.environ.get("EMB_TABLE_DTYPE", "bf16"))
    except Exception:
        if arch == "indirect":
            raise
        # Device-state fallback: the indirect path has proven robust even
        # right after a failed gather run.
        res, out = _kernel_indirect(x, table, "indirect")
    LAST_RESULTS = res
    return out.reshape(B, S, D)


# revision 28
# speedup vs baseline: 1.0969x; 1.0969x over previous
"""Trainium2 Bass kernel for CustomEmbedding lookup.

Reference semantics:
    table = where(is_num[:, None], sin(num_value/1000 * (arange(D)+1)), weight)
    out   = table[x]                    # x: (8, 4096) int32, table: (50000, 512) f32

Strategy (8 NeuronCores, SPMD, memory-bound):
  - Host: materialize the merged static table (constant at init) and keep a
    bf16 copy (tolerance is 2e-2; bf16 rounding is ~2e-3 of max).
  - Shard x across the 8 cores by batch row (4096 tokens/core); replicate
    the 50 MB bf16 table into each core's HBM.
  - Device (per core), default arch "gb16": host-compacted int16 index
    streams (lo/hi halves of the vocab, since dma_gather indices are int16),
    chunked dma_gather of bf16 rows (1 KB each) into SBUF, ACT-engine
    copy-cast to f32, HWDGE stores of the compacted f32 stream. Host
    unpermutes rows back to token positions (established baseline pattern).
    Measured: ~63.8 us (vs 66.4 us for the f32 indirect baseline).
  - Measured HW facts that shape this design (from perfetto traces here):
    SDMA engines move random 2 KB gather descriptors at ~82 ns (line rate,
    not latency-bound); dma_gather desc-gen costs ~4-5 ns/row on Pool but
    pays a ~9 us mlp-ucode library reload before the first gather;
    indirect_dma_start needs no reload but costs ~11 ns/row of Pool time
    (1.4 us per 128-row instruction incl. dispatch gaps) - it is
    desc-gen-bound end to end. Stores run at line rate (~26 GB/s/engine,
    8 KB descriptors). bf16 halves the random-read bytes.
  - Fragile configs found by experiment (each hangs the device DMA path;
    keep to the proven config): dma_gather chunks with c=1 (128 rows),
    all chunks on one SWDGE queue / num_swdge_queues=1, tile-pool bufs
    above 4(gather)+3(store) (exceeds the 8 DMA completion sem lanes),
    DVE tensor_copy concurrent with gather desc-gen.
  - EMB_KERNEL_ARCH=indirect keeps the previous f32 indirect-DMA
    implementation (~66 us); kernel() also auto-falls-back to it if the
    gather path raises.
  - EMB_TABLE_DTYPE=fp8 gathers a float8_e4m3 table (512 B rows) and
    patches the ~1k worst-quantized rows' tokens exactly on the host.
"""

import os

import numpy as np

# Problem shape (hardcoded per harness contract).
N_CORES = 8
B, S = 8, 4096          # x shape
V, D = 50000, 512       # table shape
P = 128                 # SBUF partitions
S_CORE = (B * S) // N_CORES   # tokens per core = 4096
T = S_CORE // P         # tokens per partition = 32
HALF = 32768            # int16-addressable row limit

# Static capacities for the two compacted streams (multiples of 128).
# Uniform x: nLo ~ B(4096, .655) => mean 2685, sigma ~30. Caps clear the
# mean by >4 sigma; a host-side fallback handles any overflow exactly.
LO_CAP = 2816
HI_CAP = 1536
CHUNK = 512             # rows per dma_gather

_PROGS = {}
LAST_RESULTS = None  # BassKernelResults of the last run (for test harness)
TRACE = False


def _install_ntff_hook():
    """Provide antenv.axon_hooks (absent on this image) so
    run_bass_kernel_spmd(trace=True) can capture NTFF profiles."""
    import sys
    import types

    if "antenv.axon_hooks" in sys.modules:
        return
    mod = types.ModuleType("antenv.axon_hooks")
    state = {"hook": None}
    mod.set_axon_ntff_profile_hook = lambda h: state.update(hook=h)
    mod.get_axon_ntff_profile_hook = lambda: state["hook"]
    sys.modules["antenv.axon_hooks"] = mod
    import antenv

    antenv.axon_hooks = mod
    from trn_agent_boot.trn_boot import _ntff_profile_via_ctypes

    mod.set_axon_ntff_profile_hook(
        _ntff_profile_via_ctypes("/opt/axon/libaxon_pjrt.so"))


def _chunks_sized(sizes):
    out = []
    base = 0
    for n in sizes:
        out.append((base, n))
        base += n
    return out


# Ramped chunk sizes: small first chunks complete quickly (prime the
# cast/store pipeline through the 4-queue packet round-robin), big
# steady-state chunks amortize desc-gen, small tail drains fast.
LO_SIZES = [512, 512, 512, 512, 512, 256]   # = LO_CAP
HI_SIZES = [512, 512, 512]                  # = HI_CAP


def _build_nc_gb16(tdt_name="bf16"):
    """Quantized table, chunked dma_gather -> ACT/DVE cast -> HWDGE store."""
    import concourse.bacc as bacc
    import concourse.mybir as mybir
    import concourse.tile as tile

    tdt = mybir.dt.bfloat16 if tdt_name == "bf16" else mybir.dt.float8e4
    nc = bacc.Bacc("TRN2", target_bir_lowering=False, debug=False,
                   num_devices=N_CORES, num_swdge_queues=4)
    table = nc.dram_tensor("table", [V, D], tdt,
                           kind="ExternalInput").ap()
    idx_lo = nc.dram_tensor("idxLo", [P, LO_CAP // 16], mybir.dt.int16,
                            kind="ExternalInput").ap()
    idx_hi = nc.dram_tensor("idxHi", [P, HI_CAP // 16], mybir.dt.int16,
                            kind="ExternalInput").ap()
    out_lo = nc.dram_tensor("outLo", [LO_CAP, D], mybir.dt.float32,
                            kind="ExternalOutput").ap()
    out_hi = nc.dram_tensor("outHi", [HI_CAP, D], mybir.dt.float32,
                            kind="ExternalOutput").ap()
    # DRAM row p*C + c holds stream element c*128 + p (dma_gather layout);
    # per-partition rows are contiguous so store descriptors are big.
    lo_v = out_lo.rearrange("(p c) d -> p c d", p=P)
    hi_v = out_hi.rearrange("(p c) d -> p c d", p=P)

    assert sum(LO_SIZES) == LO_CAP and sum(HI_SIZES) == HI_CAP
    chunks = ([("lo", b, n) for b, n in _chunks_sized(LO_SIZES)]
              + [("hi", b, n) for b, n in _chunks_sized(HI_SIZES)])

    with tile.TileContext(nc) as tc:
        with tc.tile_pool(name="idx", bufs=1) as idxp, \
             tc.tile_pool(name="bfrows", bufs=4) as bfp, \
             tc.tile_pool(name="f32rows", bufs=3) as f32p:
            lo_sb = idxp.tile([P, LO_CAP // 16], mybir.dt.int16, tag="ilo")
            hi_sb = idxp.tile([P, HI_CAP // 16], mybir.dt.int16, tag="ihi")
            nc.sync.dma_start(out=lo_sb[:], in_=idx_lo[:, :])
            nc.scalar.dma_start(out=hi_sb[:], in_=idx_hi[:, :])
            for k, (kind, cbase, n) in enumerate(chunks):
                src = table[:HALF, :] if kind == "lo" else table[HALF:, :]
                isb = lo_sb if kind == "lo" else hi_sb
                odr = lo_v if kind == "lo" else hi_v
                c = n // P
                rows_bf = bfp.tile([P, c * D], mybir.dt.bfloat16, tag="bf")
                nc.gpsimd.dma_gather(
                    out_ap=rows_bf[:].rearrange("p (c d) -> p c d", d=D),
                    in_ap=src,
                    idxs_ap=isb[:, cbase // 16:(cbase + n) // 16],
                    num_idxs=n,
                    num_idxs_reg=n,
                    elem_size=D,
                    single_packet=True,
                    queue_num=k % 4,
                )
                rows_f = f32p.tile([P, c * D], mybir.dt.float32, tag="f32")
                # Cast in halves so downstream stores start sooner.
                hw = max(1, c // 2) * D
                for h in range(0, c * D, hw):
                    nc.scalar.copy(out=rows_f[:, h:h + hw],
                                   in_=rows_bf[:, h:h + hw])
                nc.sync.dma_start(
                    out=odr[:, cbase // P:(cbase + n) // P, :],
                    in_=rows_f[:].rearrange("p (c d) -> p c d", d=D),
                )
    nc.compile()
    return nc


def _build_nc_indirect():
    """Fallback: 32x int32 indirect DMAs (one index per partition each)."""
    import concourse.bacc as bacc
    import concourse.bass as bass
    import concourse.mybir as mybir
    import concourse.tile as tile

    nc = bacc.Bacc("TRN2", target_bir_lowering=False, debug=False,
                   num_devices=N_CORES)
    xs = nc.dram_tensor("xs", [S_CORE], mybir.dt.int32,
                        kind="ExternalInput").ap()
    table = nc.dram_tensor("table", [V, D], mybir.dt.float32,
                           kind="ExternalInput").ap()
    out = nc.dram_tensor("out", [S_CORE, D], mybir.dt.float32,
                         kind="ExternalOutput").ap()

    GW = 4
    NW = T // GW
    with tile.TileContext(nc) as tc:
        with tc.tile_pool(name="idx", bufs=1) as idxp, \
             tc.tile_pool(name="rows", bufs=4) as rowp:
            xv = xs.rearrange("(p t) -> p t", p=P)
            idx_sb = idxp.tile([P, T], mybir.dt.int32)
            nc.sync.dma_start(out=idx_sb[:, :GW], in_=xv[:, :GW])
            nc.scalar.dma_start(out=idx_sb[:, GW:], in_=xv[:, GW:])
            outv = out.rearrange("(p t) d -> p t d", p=P)
            for w in range(NW):
                rows = rowp.tile([P, GW * D], mybir.dt.float32)
                for j in range(GW):
                    t = w * GW + j
                    nc.gpsimd.indirect_dma_start(
                        out=rows[:, j * D:(j + 1) * D],
                        out_offset=None,
                        in_=table[:],
                        in_offset=bass.IndirectOffsetOnAxis(
                            ap=idx_sb[:, t:t + 1], axis=0),
                    )
                if w < NW - 1:
                    eng = nc.sync if w % 2 == 0 else nc.scalar
                    eng.dma_start(
                        out=outv[:, w * GW:(w + 1) * GW, :],
                        in_=rows[:].rearrange("p (t d) -> p t d", d=D),
                    )
                else:
                    for j in range(GW):
                        t = w * GW + j
                        eng = nc.sync if j % 2 == 0 else nc.scalar
                        eng.dma_start(
                            out=outv[:, t, :],
                            in_=rows[:, j * D:(j + 1) * D],
                        )
    nc.compile()
    return nc


def _get_prog(arch):
    if arch not in _PROGS:
        if arch.startswith("gb16"):
            _PROGS[arch] = _build_nc_gb16(arch.split("-")[1])
        else:
            _PROGS[arch] = _build_nc_indirect()
    return _PROGS[arch]


def _merged_table(weight, num_value, is_num):
    """Merged static table: sinusoid rows where is_num, else weight."""
    table = np.array(weight, dtype=np.float32, copy=True)
    rows = np.nonzero(np.asarray(is_num))[0]
    if rows.size:
        freqs = np.arange(1, D + 1, dtype=np.float32)
        scaled = np.asarray(num_value)[rows].astype(np.float32) / np.float32(1000.0)
        table[rows] = np.sin(scaled[:, None] * freqs[None, :]).astype(np.float32)
    return table


def _wrap16(stream, cap):
    """stream (cap,) int16 -> [128, cap/16]: index i at [i%16, i//16],
    replicated across the 8 GpSimd core partition groups."""
    t = np.ascontiguousarray(stream.reshape(cap // 16, 16).T)
    return np.tile(t, (8, 1))


def _kernel_gb16(x, table, tdt_name="bf16"):
    import ml_dtypes
    from concourse.bass_utils import run_bass_kernel_spmd

    nc = _get_prog("gb16-" + tdt_name)
    np_tdt = (ml_dtypes.bfloat16 if tdt_name == "bf16"
              else ml_dtypes.float8_e4m3)
    table_bf = table.astype(np_tdt)
    xs = np.asarray(x, dtype=np.int32).reshape(N_CORES, S_CORE)
    # Rows whose quantization error is too large get patched exactly on
    # the host after the device pass (rare-row exception path).
    bad_rows = None
    if tdt_name == "fp8":
        err_row = np.abs(table - table_bf.astype(np.float32)).max(axis=1)
        denom = np.abs(table[np.unique(xs)]).max()
        bad = err_row > 0.8 * 0.02 * denom
        bad_rows = np.zeros(V, dtype=bool)
        bad_rows[np.nonzero(bad)[0]] = True
    in_maps = []
    pos = []
    for c in range(N_CORES):
        xc = xs[c]
        lo_pos = np.nonzero(xc < HALF)[0]
        hi_pos = np.nonzero(xc >= HALF)[0]
        pos.append((lo_pos, hi_pos))
        s_lo = np.full(LO_CAP, -1, dtype=np.int16)
        s_hi = np.full(HI_CAP, -1, dtype=np.int16)
        n_lo = min(lo_pos.size, LO_CAP)
        n_hi = min(hi_pos.size, HI_CAP)
        s_lo[:n_lo] = xc[lo_pos[:n_lo]].astype(np.int16)
        s_hi[:n_hi] = (xc[hi_pos[:n_hi]] - HALF).astype(np.int16)
        in_maps.append({"table": table_bf,
                        "idxLo": _wrap16(s_lo, LO_CAP),
                        "idxHi": _wrap16(s_hi, HI_CAP)})

    res = run_bass_kernel_spmd(nc, in_maps, core_ids=list(range(N_CORES)),
                               trace=TRACE)
    out = np.empty((N_CORES, S_CORE, D), dtype=np.float32)
    for c in range(N_CORES):
        lo_pos, hi_pos = pos[c]
        r = res.results[c]
        n_lo = min(lo_pos.size, LO_CAP)
        n_hi = min(hi_pos.size, HI_CAP)
        # DRAM row p*C + c holds stream element c*128 + p: transpose back.
        lo_rows = r["outLo"].reshape(P, LO_CAP // P, D).transpose(1, 0, 2)
        hi_rows = r["outHi"].reshape(P, HI_CAP // P, D).transpose(1, 0, 2)
        out[c][lo_pos[:n_lo]] = lo_rows.reshape(LO_CAP, D)[:n_lo]
        out[c][hi_pos[:n_hi]] = hi_rows.reshape(HI_CAP, D)[:n_hi]
        # Exact host fallback for (statistically impossible) cap overflow.
        for ps, n_cap in ((lo_pos, n_lo), (hi_pos, n_hi)):
            if ps.size > n_cap:
                ovf = ps[n_cap:]
                out[c][ovf] = table[xs[c][ovf]]
        if bad_rows is not None:
            pos_bad = np.nonzero(bad_rows[xs[c]])[0]
            if pos_bad.size:
                out[c][pos_bad] = table[xs[c][pos_bad]]
    return res, out


def _kernel_indirect(x, table, arch="indirect"):
    from concourse.bass_utils import run_bass_kernel_spmd

    nc = _get_prog(arch)
    xflat = np.ascontiguousarray(np.asarray(x, dtype=np.int32).reshape(-1))
    in_maps = [
        {"xs": xflat[c * S_CORE:(c + 1) * S_CORE], "table": table}
        for c in range(N_CORES)
    ]
    res = run_bass_kernel_spmd(nc, in_maps, core_ids=list(range(N_CORES)),
                               trace=TRACE)
    out = np.stack([r["out"] for r in res.results])
    return res, out


def kernel(x, weight, num_value, is_num):
    global LAST_RESULTS
    if TRACE:
        _install_ntff_hook()

    table = _merged_table(weight, num_value, is_num)
    arch = os.environ.get("EMB_KERNEL_ARCH", "gb16")
    try:
        if arch == "indirect":
            res, out = _kernel_indirect(x, table, arch)
        else:
            res, out = _kernel_gb16(x, table,
                                    os.environ.get("EMB_TABLE_DTYPE", "bf16"))
    except Exception:
        if arch == "indirect":
            raise
        # Device-state fallback: the indirect path has proven robust even
        # right after a failed gather run.
        res, out = _kernel_indirect(x, table, "indirect")
    LAST_RESULTS = res
    return out.reshape(B, S, D)
